# revision 20
# baseline (speedup 1.0000x reference)
"""LATTE GNN message passing on 8 trn2 cores (v3).

Design (v3, from v2 baseline):
- Edges sharded by dst node: core k owns dst nodes [6272k, 6272(k+1)).
- Table rows shrink 512B -> 256B: h stored fp8e4m3 (packed in bf16 cols
  0..63), aj (16 bf16, all metapaths) at cols 64..79. Stage-2 gathers
  therefore move half the bytes of v2.
- feats/W/FOWNP are fp8 (halves stage-1 HBM reads; fp8 PE matmuls).
- Stage 2 is processed in SEGMENTS of whole blocks (<= SEGT tiles): one
  gather chunk buffer per segment, ONE DVE op per segment for logit-add /
  prelu / exp / msg-mult / s-copy (vs per-block ops in v2), one strided
  reduce_sum per block. Softmax weights are broadcast via stride-0 APs
  (no 128-wide exp materialization).
- ERAW is bf16 512B rows (132 of 256 cols used); pass results written as
  ONE DMA per pass. Realign gathers (per metapath) merge chunk A+B and
  normalize by s-sums directly into persistent SBUF accumulators - EALN
  and the v2 per-metapath DRAM round trip are gone.
- Stage 3 is batched 4 blocks per emission with 4D APs, reading relation
  embeddings straight from the SBUF accumulators.
- Segment-tail gather slots use idx -1 (trailing negatives are skipped by
  the SWDGE ucode - no DMA cost); in-block pad slots still point at the
  pad rows (h=0, aj=-100).
"""

import sys
import numpy as np

sys.path.insert(0, "/root/shadow")
try:
    import setup_ntff  # noqa: F401
except Exception:
    pass
sys.path.insert(0, "/opt/trn_rl_repo")

import ml_dtypes
import concourse.bass as bass
import concourse.bacc as bacc
import concourse.mybir as mybir
import concourse.tile as tile
from concourse.bass_utils import run_bass_kernel_spmd
from concourse.library_config import mlp

N = 50000
E = 800000
M = 4
NP = 2 * M               # passes: (metapath, src-chunk)
H = 4
C = 32
IN = 256
D = 128
NCORES = 8
CORE_N = 6272
NB = 49
NT = 50176
CHA = 32767              # nodes 0..32766 -> chunk A (row = node+1)
TBAR = 32768             # TBA rows (row 0 = pad)
NBROWS = NT + 2 - TBAR   # 17410 chunk-B rows (last = pad)
PADB = NBROWS - 1
GSZ = 1024               # slots per dma_gather
GT = GSZ // 128          # 8 tiles per gather
GW = GSZ // 16           # 64 idx cols per gather
EW = 128                 # table row: 128 bf16 = 256B
SEGT = 48                # max tiles per stage-2 segment rectangle
NCHO = -(-CORE_N // GSZ)  # 7 realign gathers per stream
F32 = mybir.dt.float32
BF16 = mybir.dt.bfloat16
F8 = mybir.dt.float8e4
I16 = mybir.dt.int16
AF = mybir.ActivationFunctionType
OP = mybir.AluOpType
FP8NP = ml_dtypes.float8_e4m3

_CACHE = {}


# ---------------------------------------------------------------- host prep

def _wrap_chunks(stream):
    """idx stream (len multiple of 16) -> wrapped [128, len/16] int16."""
    n = len(stream)
    a = np.asarray(stream, dtype=np.int16).reshape(n // 16, 16).T
    return np.tile(a, (8, 1))


def _plan_and_pack(edge_index):
    ei = np.asarray(edge_index)
    ranks = {}
    perms = {}
    edges = {}
    Tk = np.zeros((NCORES, NP, NB), dtype=np.int64)
    for k in range(NCORES):
        for m in range(M):
            src = ei[m, 0].astype(np.int64)
            dst = ei[m, 1].astype(np.int64)
            sel = (dst // CORE_N) == k
            ls = dst[sel] - CORE_N * k
            sr = src[sel]
            isA = sr <= CHA - 1
            for P, mask in ((2 * m, isA), (2 * m + 1, ~isA)):
                lw, sw = ls[mask], sr[mask]
                deg = np.bincount(lw, minlength=CORE_N)
                perm = np.argsort(-deg, kind="stable")
                rank = np.empty(CORE_N, dtype=np.int64)
                rank[perm] = np.arange(CORE_N)
                perms[(k, P)] = perm
                ranks[(k, P)] = rank
                edges[(k, P)] = (lw, sw, rank)
                Tk[k, P] = deg[perm].reshape(NB, 128).max(axis=1)

    Tu = Tk.max(axis=0)                       # [NP, NB]
    # rectangular segments: nb consecutive blocks padded to the first
    # block's T (blocks are degree-sorted desc), nb*Tmax <= SEGT tiles.
    # Tile order within a segment is LAYER-major: tile(o, j) = o*nb + j,
    # so the per-block sums reduce via log2(Tmax) contiguous in-place
    # folds instead of per-block strided reduces.
    segs = []          # per P: [(b0, nb, Tmax, padbase, rect)]
    zeros = []         # per P: (b0, count) of all-zero tail blocks
    ngath = []
    segb0 = np.zeros((NP, NB), dtype=np.int64)
    segnb = np.zeros((NP, NB), dtype=np.int64)
    segpb = np.zeros((NP, NB), dtype=np.int64)
    for P in range(NP):
        segP = []
        padbase = 0
        b = 0
        z0 = NB
        while b < NB:
            Tmax = int(Tu[P][b])
            if Tmax == 0:
                z0 = b
                break
            nb = 1
            while (b + nb < NB and int(Tu[P][b + nb]) > 0
                   and (nb + 1) * Tmax <= SEGT):
                nb += 1
            rect = nb * Tmax
            segP.append((b, nb, Tmax, padbase, rect))
            segb0[P][b:b + nb] = b
            segnb[P][b:b + nb] = nb
            segpb[P][b:b + nb] = padbase
            padbase += GT * (-(-rect // GT))
            b += nb
        zeros.append((z0, NB - z0))
        segs.append(segP)
        ngath.append(padbase // GT)

    plan = {"Tu": Tu, "segs": segs, "zeros": zeros, "ngath": ngath,
            "perms": perms}

    percore = []
    for k in range(NCORES):
        cols = []
        for P in range(NP):
            S = ngath[P] * GSZ
            stream = np.full(S, -1, dtype=np.int64)
            padval = 0 if P % 2 == 0 else PADB
            for (b0, nb, Tmax, pb, rect) in segs[P]:
                stream[pb * 128:(pb + rect) * 128] = padval
            lw, sw, rank = edges[(k, P)]
            lane = rank[lw]
            order = np.argsort(lane, kind="stable")
            lw2, sw2 = lane[order], sw[order]
            first = np.searchsorted(lw2, lw2)
            occ = np.arange(len(lw2)) - first
            b = lw2 // 128
            p = lw2 % 128
            tilei = segpb[P][b] + occ * segnb[P][b] + (b - segb0[P][b])
            slot = tilei * 128 + p
            stream[slot] = (sw2 + 1) if P % 2 == 0 else (sw2 - CHA)
            cols.append(_wrap_chunks(stream))
        for P in range(NP):
            s = np.full(NCHO * GSZ, -1, dtype=np.int64)
            s[:CORE_N] = ranks[(k, P)]
            cols.append(_wrap_chunks(s))
        percore.append(np.ascontiguousarray(np.concatenate(cols, axis=1)))
    plan["idx_cols"] = percore[0].shape[1]
    return plan, percore


def _prep_host(feats, edge_index, W, attn, rel_attn_l, rel_attn_r,
               rel_attn_bias):
    plan, idxw = _plan_and_pack(edge_index)

    featsT = np.zeros((IN, NT), dtype=np.float32)
    featsT[:, :N] = np.asarray(feats, dtype=np.float32).T
    featsT = featsT.astype(ml_dtypes.bfloat16)
    Wb = np.asarray(W, dtype=np.float32).astype(ml_dtypes.bfloat16)
    attn = np.asarray(attn, dtype=np.float32)
    Acat = np.zeros((D, 32), dtype=np.float32)
    for m in range(M):
        for h in range(H):
            Acat[h * C:(h + 1) * C, m * H + h] = attn[m, h, C:]       # aj
            Acat[h * C:(h + 1) * C, 16 + m * H + h] = attn[m, h, :C]  # ai
    Acat = Acat.astype(ml_dtypes.bfloat16)
    ident = np.eye(128, dtype=np.float32).astype(ml_dtypes.bfloat16)
    padrow = np.zeros((1, EW), dtype=np.float32)
    padrow[0, 64:80] = -100.0
    padrow = padrow.astype(ml_dtypes.bfloat16)
    rel_l = np.asarray(rel_attn_l, dtype=np.float32).reshape(1, 128)
    rel_r = np.asarray(rel_attn_r, dtype=np.float32).reshape(1, 640)
    relb = np.repeat(np.asarray(rel_attn_bias, np.float32), 4).reshape(1, 20)
    zer = np.zeros((1, 132), dtype=np.float32).astype(ml_dtypes.bfloat16)

    ft = np.asarray(feats, np.float32).T
    in_maps = []
    for k in range(NCORES):
        fown = np.zeros((IN, CORE_N), dtype=np.float32)
        lo = CORE_N * k
        hi = min(N, lo + CORE_N)
        fown[:, :hi - lo] = ft[:, lo:hi]
        fownp = np.empty((IN, NP * CORE_N), dtype=np.float32)
        for P in range(NP):
            fownp[:, P * CORE_N:(P + 1) * CORE_N] = \
                fown[:, plan["perms"][(k, P)]]
        in_maps.append({
            "featsT": featsT, "FOWN": fown.astype(ml_dtypes.bfloat16),
            "FOWNP": fownp.astype(ml_dtypes.bfloat16),
            "Wb": Wb, "ACAT": Acat, "IDENT": ident, "PADR": padrow,
            "RELL": rel_l, "RELR": rel_r, "RELB": relb, "ZERB": zer,
            "IDXW": idxw[k],
        })
    return plan, in_maps


# ---------------------------------------------------------------- device

def _build(plan):
    nc = bacc.Bacc("TRN2", target_bir_lowering=False, debug=False,
                   num_swdge_queues=4)
    ds = bass.ds

    featsT = nc.dram_tensor("featsT", [IN, NT], BF16, kind="ExternalInput")
    FOWN = nc.dram_tensor("FOWN", [IN, CORE_N], BF16, kind="ExternalInput")
    FOWNP = nc.dram_tensor("FOWNP", [IN, NP * CORE_N], BF16,
                           kind="ExternalInput")
    Wb = nc.dram_tensor("Wb", [IN, D], BF16, kind="ExternalInput")
    ACAT = nc.dram_tensor("ACAT", [D, 32], BF16, kind="ExternalInput")
    IDENT = nc.dram_tensor("IDENT", [128, 128], BF16, kind="ExternalInput")
    PADR = nc.dram_tensor("PADR", [1, EW], BF16, kind="ExternalInput")
    RELL = nc.dram_tensor("RELL", [1, 128], F32, kind="ExternalInput")
    RELR = nc.dram_tensor("RELR", [1, 640], F32, kind="ExternalInput")
    RELB = nc.dram_tensor("RELB", [1, 20], F32, kind="ExternalInput")
    ZERB = nc.dram_tensor("ZERB", [1, 132], BF16, kind="ExternalInput")
    IDXW = nc.dram_tensor("IDXW", [128, plan["idx_cols"]], I16,
                          kind="ExternalInput")

    TBA = nc.dram_tensor("TBA", [TBAR, EW], BF16)
    TBB = nc.dram_tensor("TBB", [NBROWS, EW], BF16)
    OWNT = nc.dram_tensor("OWNT", [CORE_N, 128], BF16)
    ERAW = nc.dram_tensor("ERAW", [NP * CORE_N, 256], BF16)
    OUT = nc.dram_tensor("OUT", [CORE_N, D], F32, kind="ExternalOutput")

    Tu = plan["Tu"]
    segs = plan["segs"]
    zeros = plan["zeros"]
    ngath = plan["ngath"]
    aln_off = sum(ngath) * GW

    qn = [0]

    def nextq():
        q = qn[0] % 4
        qn[0] += 1
        return q

    with tile.TileContext(nc) as tc, \
         nc.allow_low_precision(reason="bf16 partial sums within 2e-2 tol"):
        with tc.tile_pool(name="const", bufs=1) as cp:
            nc.gpsimd.load_library(mlp)
            W0 = cp.tile([128, 128], BF16)
            nc.sync.dma_start(out=W0[:], in_=Wb[0:128, :])
            W1 = cp.tile([128, 128], BF16)
            nc.sync.dma_start(out=W1[:], in_=Wb[128:256, :])
            Ac = cp.tile([128, 32], BF16)
            nc.sync.dma_start(out=Ac[:], in_=ACAT[:])
            idn = cp.tile([128, 128], BF16)
            nc.sync.dma_start(out=idn[:], in_=IDENT[:])
            rlr = cp.tile([128, 128], F32)
            nc.sync.dma_start(out=rlr[:], in_=RELL[:].to_broadcast((128, 128)))
            rrr = cp.tile([128, 640], F32)
            nc.sync.dma_start(out=rrr[:], in_=RELR[:].to_broadcast((128, 640)))
            rbb = cp.tile([128, 20], F32)
            nc.sync.dma_start(out=rbb[:], in_=RELB[:].to_broadcast((128, 20)))
            zrow = cp.tile([128, 132], BF16)
            nc.sync.dma_start(out=zrow[:], in_=ZERB[:].to_broadcast((128, 132)))
            rlrb = cp.tile([128, 128], BF16)
            nc.vector.tensor_copy(out=rlrb[:], in_=rlr[:])
            rrrb = cp.tile([128, 640], BF16)
            nc.vector.tensor_copy(out=rrrb[:], in_=rrr[:])
            pr0 = cp.tile([1, EW], BF16)
            nc.sync.dma_start(out=pr0[:], in_=PADR[:])
            nc.sync.dma_start(out=TBA[0:1, :], in_=pr0[:])
            nc.sync.dma_start(out=TBB[NBROWS - 1:NBROWS, :], in_=pr0[:])

            with tc.tile_pool(name="s1", bufs=3) as p1, \
                 tc.tile_pool(name="s1p", bufs=2, space="PSUM") as pp1, \
                 tc.tile_pool(name="aib", bufs=1) as pa, \
                 tc.tile_pool(name="prj", bufs=2) as pprj, \
                 tc.tile_pool(name="pix", bufs=2) as pix, \
                 tc.tile_pool(name="hgp", bufs=3) as pg, \
                 tc.tile_pool(name="msg", bufs=1) as pms, \
                 tc.tile_pool(name="sml", bufs=2) as psm, \
                 tc.tile_pool(name="osb", bufs=1) as posb, \
                 tc.tile_pool(name="acc", bufs=1) as pacc, \
                 tc.tile_pool(name="rlp", bufs=1) as prl, \
                 tc.tile_pool(name="s3", bufs=1) as p3, \
                 tc.tile_pool(name="s3h", bufs=2) as p3h:

                # ---- stage 1: global table projection (fp8 h + bf16 aj) ----
                def project_tbl():
                    nsteps = -(-NT // 512)
                    for i in range(nsteps):
                        w = min(512, NT - i * 512)
                        ng = w // 128
                        fa = p1.tile([128, 512], BF16, tag="fa")
                        nc.scalar.dma_start(out=fa[:, 0:w],
                                            in_=featsT[0:128, ds(i * 512, w)])
                        fb = p1.tile([128, 512], BF16, tag="fb")
                        nc.scalar.dma_start(out=fb[:, 0:w],
                                            in_=featsT[128:256,
                                                       ds(i * 512, w)])
                        hrow = p1.tile([128, 4 * 80], BF16, tag="hrow")
                        hp = pp1.tile([128, 512], F32, tag="hp")
                        for j in range(ng):
                            nc.tensor.matmul(out=hp[:, j * 128:(j + 1) * 128],
                                             lhsT=fa[:, j * 128:(j + 1) * 128],
                                             rhs=W0[:], start=True, stop=False)
                            nc.tensor.matmul(out=hp[:, j * 128:(j + 1) * 128],
                                             lhsT=fb[:, j * 128:(j + 1) * 128],
                                             rhs=W1[:], start=False, stop=True)
                        hrs = p1.tile([128, 512], BF16, tag="hrs")
                        nc.scalar.activation(hrs[:, 0:w], hp[:, 0:w], AF.Relu)
                        # fp8 h -> hrow cols [j*80, j*80+64) (bitcast view)
                        nc.scalar.activation(
                            hrow[:, 0:ng * 80].bitcast(F8).rearrange(
                                "p (g c) -> p g c", c=160)[:, :, 0:128],
                            hp[:, 0:w].rearrange("p (g c) -> p g c", c=128),
                            AF.Relu)
                        htp = pp1.tile([128, 512], BF16, tag="htp")
                        for j in range(ng):
                            nc.tensor.transpose(
                                out=htp[:, j * 128:(j + 1) * 128],
                                in_=hrs[:, j * 128:(j + 1) * 128],
                                identity=idn[:])
                        hts = p1.tile([128, 512], BF16, tag="hts")
                        nc.scalar.activation(hts[:, 0:w], htp[:, 0:w], AF.Copy)
                        ap2 = pp1.tile([128, 64], F32, tag="ap2")
                        for j in range(ng):
                            nc.tensor.matmul(out=ap2[:, j * 16:(j + 1) * 16],
                                             lhsT=hts[:, j * 128:(j + 1) * 128],
                                             rhs=Ac[:, 0:16],
                                             start=True, stop=True)
                        nc.vector.tensor_copy(
                            out=hrow[:, 0:ng * 80].rearrange(
                                "p (g c) -> p g c", c=80)[:, :, 64:80],
                            in_=ap2[:, 0:ng * 16].rearrange(
                                "p (g c) -> p g c", c=16))
                        r0 = 1 + i * 512

                        def wout(tgt, row, g0, p0, np_, cnt=1):
                            nc.scalar.dma_start(
                                out=tgt[ds(row, cnt * np_), 0:80].rearrange(
                                    "(g p) c -> p g c", p=np_),
                                in_=hrow[p0:p0 + np_,
                                         g0 * 80:(g0 + cnt) * 80].rearrange(
                                    "p (g c) -> p g c", c=80))

                        if r0 + w <= TBAR:
                            wout(TBA, r0, 0, 0, 128, ng)
                        elif r0 >= TBAR:
                            wout(TBB, r0 - TBAR, 0, 0, 128, ng)
                        else:
                            nA = TBAR - r0
                            gA, pA = nA // 128, nA % 128
                            if gA > 0:
                                wout(TBA, r0, 0, 0, 128, gA)
                            if pA > 0:
                                wout(TBA, r0 + gA * 128, gA, 0, pA)
                                wout(TBB, 0, gA, pA, 128 - pA)
                            bstart = gA + (1 if pA else 0)
                            boff = bstart * 128 - nA
                            if bstart < ng:
                                wout(TBB, boff, bstart, 0, 128, ng - bstart)

                def project_own():
                    nsteps = -(-CORE_N // 512)
                    for i in range(nsteps):
                        w = min(512, CORE_N - i * 512)
                        ng = w // 128
                        fa = p1.tile([128, 512], BF16, tag="fa")
                        nc.scalar.dma_start(out=fa[:, 0:w],
                                            in_=FOWN[0:128, ds(i * 512, w)])
                        fb = p1.tile([128, 512], BF16, tag="fb")
                        nc.scalar.dma_start(out=fb[:, 0:w],
                                            in_=FOWN[128:256,
                                                     ds(i * 512, w)])
                        hp = pp1.tile([128, 512], F32, tag="hp")
                        for j in range(ng):
                            nc.tensor.matmul(out=hp[:, j * 128:(j + 1) * 128],
                                             lhsT=fa[:, j * 128:(j + 1) * 128],
                                             rhs=W0[:], start=True, stop=False)
                            nc.tensor.matmul(out=hp[:, j * 128:(j + 1) * 128],
                                             lhsT=fb[:, j * 128:(j + 1) * 128],
                                             rhs=W1[:], start=False, stop=True)
                        ohr = p1.tile([128, 512], BF16, tag="ohr")
                        nc.scalar.activation(ohr[:, 0:w], hp[:, 0:w], AF.Relu)
                        nc.scalar.dma_start(
                            out=OWNT[ds(i * 512, w), :].rearrange(
                                "(g p) c -> p g c", p=128),
                            in_=ohr[:, 0:w].rearrange("p (g c) -> p g c",
                                                      c=128))

                def emit_aibs(P, blockai):
                    m = P // 2
                    base = P * CORE_N
                    nsteps = -(-CORE_N // 512)
                    for i in range(nsteps):
                        w = min(512, CORE_N - i * 512)
                        ng = w // 128
                        fa = pprj.tile([128, 512], BF16, tag="pfa")
                        nc.scalar.dma_start(
                            out=fa[:, 0:w],
                            in_=FOWNP[0:128, ds(base + i * 512, w)])
                        fb = pprj.tile([128, 512], BF16, tag="pfb")
                        nc.scalar.dma_start(
                            out=fb[:, 0:w],
                            in_=FOWNP[128:256, ds(base + i * 512, w)])
                        hp = pp1.tile([128, 512], F32, tag="hp")
                        for j in range(ng):
                            nc.tensor.matmul(out=hp[:, j * 128:(j + 1) * 128],
                                             lhsT=fa[:, j * 128:(j + 1) * 128],
                                             rhs=W0[:], start=True, stop=False)
                            nc.tensor.matmul(out=hp[:, j * 128:(j + 1) * 128],
                                             lhsT=fb[:, j * 128:(j + 1) * 128],
                                             rhs=W1[:], start=False, stop=True)
                        hrs = pprj.tile([128, 512], BF16, tag="phr")
                        nc.scalar.activation(hrs[:, 0:w], hp[:, 0:w], AF.Relu)
                        htp = pp1.tile([128, 512], BF16, tag="htp")
                        for j in range(ng):
                            nc.tensor.transpose(
                                out=htp[:, j * 128:(j + 1) * 128],
                                in_=hrs[:, j * 128:(j + 1) * 128],
                                identity=idn[:])
                        hts = pprj.tile([128, 512], BF16, tag="pts")
                        nc.scalar.activation(hts[:, 0:w], htp[:, 0:w], AF.Copy)
                        ap4 = pp1.tile([128, 64], F32, tag="ap2")
                        for j in range(ng):
                            nc.tensor.matmul(
                                out=ap4[:, j * 4:(j + 1) * 4],
                                lhsT=hts[:, j * 128:(j + 1) * 128],
                                rhs=Ac[:, 16 + 4 * m:20 + 4 * m],
                                start=True, stop=True)
                        nc.vector.tensor_copy(
                            out=blockai[:, i * 16:i * 16 + ng * 4],
                            in_=ap4[:, 0:ng * 4])

                # ---- stage 3 (batched, nb <= 4 blocks) ----
                def emit_s3(b0, nb, ams):
                    hb = p3h.tile([128, 4 * 128], BF16, tag="hb")
                    hbv = hb[:, 0:nb * 128]
                    nc.sync.dma_start(
                        out=hbv.rearrange("p (g c) -> p g c", c=128),
                        in_=OWNT[ds(b0 * 128, nb * 128), :].rearrange(
                            "(g p) c -> p g c", p=128))

                    def rel_src(r):
                        if r < 4:
                            return ams[r][:, ds(b0 * 128, nb * 128)].rearrange(
                                "p (g c) -> p g c", c=128)
                        return hbv.rearrange("p (g c) -> p g c", c=128)

                    bl0 = p3.tile([128, 4 * 128], BF16, tag="bl0")
                    nc.vector.tensor_tensor(
                        out=bl0[:, 0:nb * 128].rearrange("p (g c) -> p g c",
                                                         c=128),
                        in0=hbv.rearrange("p (g c) -> p g c", c=128),
                        in1=rlrb[:].unsqueeze(1).to_broadcast((128, nb, 128)),
                        op=OP.mult)
                    blr = p3.tile([128, 4 * 128], BF16, tag="blr")
                    nc.scalar.activation(blr[:, 0:nb * 128],
                                         bl0[:, 0:nb * 128], AF.Relu)
                    t1 = p3.tile([128, 4 * 640], BF16, tag="t1")
                    t1v = t1[:].rearrange("p (g r d) -> p g r d", r=5, d=128)
                    for r in range(5):
                        nc.vector.tensor_tensor(
                            out=t1v[:, 0:nb, r:r + 1, :].squeeze(2),
                            in0=rel_src(r),
                            in1=rrrb[:, r * 128:(r + 1) * 128].unsqueeze(
                                1).to_broadcast((128, nb, 128)),
                            op=OP.mult)
                    t2 = p3.tile([128, 4 * 640], BF16, tag="t2")
                    nc.scalar.activation(t2[:, 0:nb * 640],
                                         t1[:, 0:nb * 640], AF.Relu)
                    t3 = p3.tile([128, 4 * 640], BF16, tag="t3")
                    nc.vector.tensor_tensor(
                        out=t3[:, 0:nb * 640].rearrange(
                            "p (g r d) -> p g r d", r=5, d=128),
                        in0=t2[:, 0:nb * 640].rearrange(
                            "p (g r d) -> p g r d", r=5, d=128),
                        in1=blr[:, 0:nb * 128].rearrange(
                            "p (g c) -> p g c", c=128).unsqueeze(
                            2).to_broadcast((128, nb, 5, 128)),
                        op=OP.mult)
                    bmat = p3.tile([128, 4 * 20], F32, tag="bmat")
                    nc.vector.reduce_sum(
                        out=bmat[:, 0:nb * 20],
                        in_=t3[:, 0:nb * 640].rearrange(
                            "p (grh c) -> p grh c", c=32),
                        axis=mybir.AxisListType.X)
                    bm2 = p3.tile([128, 4 * 20], F32, tag="bm2")
                    nc.vector.tensor_tensor(
                        out=bm2[:, 0:nb * 20].rearrange(
                            "p (g r h) -> p g r h", r=5, h=4),
                        in0=bmat[:, 0:nb * 20].rearrange(
                            "p (g r h) -> p g r h", r=5, h=4),
                        in1=rbb[:].rearrange("p (r h) -> p r h", h=4).
                        unsqueeze(1).to_broadcast((128, nb, 5, 4)),
                        op=OP.add)
                    bview = bm2[:, 0:nb * 20].rearrange(
                        "p (g r h) -> p g h r", r=5, h=4)
                    vmax = p3.tile([128, 4 * 4], F32, tag="vmax")
                    nc.vector.reduce_max(
                        out=vmax[:, 0:nb * 4], in_=bview,
                        axis=mybir.AxisListType.X)
                    eb = p3.tile([128, 4 * 20], F32, tag="eb")
                    nc.vector.tensor_tensor(
                        out=eb[:, 0:nb * 20].rearrange(
                            "p (g r h) -> p g h r", r=5, h=4),
                        in0=bview,
                        in1=vmax[:, 0:nb * 4].rearrange(
                            "p (g h) -> p g h", h=4).unsqueeze(
                            3).to_broadcast((128, nb, 4, 5)),
                        op=OP.subtract)
                    eb2 = p3.tile([128, 4 * 20], F32, tag="eb2")
                    nc.scalar.activation(eb2[:, 0:nb * 20],
                                         eb[:, 0:nb * 20], AF.Exp)
                    vs = p3.tile([128, 4 * 4], F32, tag="vs")
                    nc.vector.reduce_sum(
                        out=vs[:, 0:nb * 4],
                        in_=eb2[:, 0:nb * 20].rearrange(
                            "p (g r h) -> p g h r", r=5, h=4),
                        axis=mybir.AxisListType.X)
                    rs = p3.tile([128, 4 * 4], F32, tag="rs")
                    nc.vector.reciprocal(out=rs[:, 0:nb * 4],
                                         in_=vs[:, 0:nb * 4])
                    bw = p3.tile([128, 4 * 20], F32, tag="bw")
                    nc.vector.tensor_tensor(
                        out=bw[:, 0:nb * 20].rearrange(
                            "p (g r h) -> p g h r", r=5, h=4),
                        in0=eb2[:, 0:nb * 20].rearrange(
                            "p (g r h) -> p g h r", r=5, h=4),
                        in1=rs[:, 0:nb * 4].rearrange(
                            "p (g h) -> p g h", h=4).unsqueeze(
                            3).to_broadcast((128, nb, 4, 5)),
                        op=OP.mult)
                    tm = p3.tile([128, 4 * 640], BF16, tag="tm")
                    tmv = tm[:].rearrange("p (g r hc) -> p g r hc", r=5,
                                          hc=128)
                    bwv = bw[:, 0:nb * 20].rearrange(
                        "p (g r h) -> p g r h", r=5, h=4)
                    for r in range(5):
                        nc.vector.tensor_tensor(
                            out=tmv[:, 0:nb, r:r + 1, :].squeeze(2).rearrange(
                                "p g (h c) -> p g h c", c=32),
                            in0=rel_src(r).rearrange(
                                "p g (h c) -> p g h c", c=32),
                            in1=bwv[:, 0:nb, r:r + 1, :].squeeze(
                                2).unsqueeze(3).to_broadcast(
                                (128, nb, 4, 32)),
                            op=OP.mult)
                    acc = p3.tile([128, 4 * 128], F32, tag="acc")
                    nc.vector.reduce_sum(
                        out=acc[:, 0:nb * 128].rearrange(
                            "p (g d) -> p g d", d=128),
                        in_=tm[:, 0:nb * 640].rearrange(
                            "p (g r d) -> p g d r", r=5, d=128),
                        axis=mybir.AxisListType.X)
                    ob = p3.tile([128, 4 * 128], F32, tag="ob")
                    nc.scalar.activation(ob[:, 0:nb * 128],
                                         acc[:, 0:nb * 128], AF.Relu)
                    nc.sync.dma_start(
                        out=OUT[ds(b0 * 128, nb * 128), :].rearrange(
                            "(g p) c -> p g c", p=128),
                        in_=ob[:, 0:nb * 128].rearrange("p (g c) -> p g c",
                                                        c=128))

                # ---- emit: projection, then passes ----
                project_tbl()
                project_own()

                scol = [0]
                ams = [None] * M
                for P in range(NP):
                    m = P // 2
                    blockai = pa.tile([128, NB * 4], BF16, tag="bai")
                    emit_aibs(P, blockai)
                    src = TBA if P % 2 == 0 else TBB
                    osb = posb.tile([128, NB * 132], BF16, tag="osb")
                    for (b0, nb, Tmax, pb, rect) in segs[P]:
                        ngs = -(-rect // GT)
                        ix = pix.tile([128, (SEGT // GT + 1) * GW], I16,
                                      tag="ix")
                        nc.sync.dma_start(
                            out=ix[:, 0:ngs * GW],
                            in_=IDXW[:, ds(scol[0] + (pb // GT) * GW,
                                           ngs * GW)])
                        hg = pg.tile([128, SEGT * EW], BF16, tag="hg")
                        segq = nextq()
                        for g in range(ngs):
                            ni = min(GSZ, rect * 128 - g * GSZ)
                            nc.gpsimd.dma_gather(
                                hg[:, ds(g * GSZ, ni)].rearrange(
                                    "p (t e) -> p t e", e=EW),
                                src[:], ix[:, ds(g * GW, ni // 16)],
                                ni, ni, EW, queue_num=segq)
                        # tile (o, j) = o*nb + j; layer-major rectangle
                        lg = psm.tile([128, SEGT * 4], BF16, tag="lg")
                        nc.vector.tensor_tensor(
                            out=lg[:, 0:rect * 4].rearrange(
                                "p (o j h) -> p o j h", j=nb, h=4),
                            in0=hg[:].rearrange(
                                "p (t e) -> p t e",
                                e=EW)[:, 0:rect,
                                      64 + 4 * m:68 + 4 * m].rearrange(
                                "p (o j) h -> p o j h", j=nb),
                            in1=blockai[:, b0 * 4:(b0 + nb) * 4].rearrange(
                                "p (j h) -> p j h", h=4).unsqueeze(
                                1).to_broadcast((128, Tmax, nb, 4)),
                            op=OP.add)
                        # exp(leakyrelu(x)) == exp(0.2x) * exp(0.8*relu(x))
                        e1 = psm.tile([128, SEGT * 4], BF16, tag="e1")
                        nc.scalar.activation(e1[:, 0:rect * 4],
                                             lg[:, 0:rect * 4], AF.Exp,
                                             scale=0.2)
                        lr = psm.tile([128, SEGT * 4], BF16, tag="lr")
                        nc.scalar.activation(lr[:, 0:rect * 4],
                                             lg[:, 0:rect * 4], AF.Relu)
                        e2 = psm.tile([128, SEGT * 4], BF16, tag="e2")
                        nc.scalar.activation(e2[:, 0:rect * 4],
                                             lr[:, 0:rect * 4], AF.Exp,
                                             scale=0.8)
                        mv = pms.tile([128, SEGT * 132], BF16, tag="mv")
                        mvv = mv[:].rearrange("p (t c) -> p t c", c=132)
                        # s into mv[:, :, 128:132] (fold carries it along)
                        nc.vector.tensor_tensor(
                            out=mvv[:, 0:rect, 128:132],
                            in0=e1[:, 0:rect * 4].rearrange(
                                "p (t h) -> p t h", h=4),
                            in1=e2[:, 0:rect * 4].rearrange(
                                "p (t h) -> p t h", h=4),
                            op=OP.mult)
                        h8 = hg[:].bitcast(F8).rearrange(
                            "p (t e) -> p t e", e=2 * EW)
                        nc.vector.tensor_tensor(
                            out=mvv[:, 0:rect, 0:128].rearrange(
                                "p t (h c) -> p t h c", c=32),
                            in0=h8[:, 0:rect, 0:128].rearrange(
                                "p t (h c) -> p t h c", c=32),
                            in1=mvv[:, 0:rect, 128:132].unsqueeze(
                                3).to_broadcast((128, rect, 4, 32)),
                            op=OP.mult)
                        # fold layers: [L, nb, 132] -> [1, nb, 132]
                        osl = osb[:, b0 * 132:(b0 + nb) * 132]
                        L = Tmax
                        while L > 2:
                            keep = -(-L // 2)
                            fold = L - keep
                            nc.vector.tensor_tensor(
                                out=mv[:, 0:fold * nb * 132],
                                in0=mv[:, 0:fold * nb * 132],
                                in1=mv[:, ds(keep * nb * 132,
                                             fold * nb * 132)],
                                op=OP.add)
                            L = keep
                        if L == 2:
                            nc.vector.tensor_tensor(
                                out=osl, in0=mv[:, 0:nb * 132],
                                in1=mv[:, ds(nb * 132, nb * 132)],
                                op=OP.add)
                        else:
                            nc.vector.tensor_copy(out=osl,
                                                  in_=mv[:, 0:nb * 132])
                    z0, nz = zeros[P]
                    if nz > 0:
                        nc.vector.tensor_copy(
                            out=osb[:, ds(z0 * 132, nz * 132)].rearrange(
                                "p (z c) -> p z c", c=132),
                            in_=zrow[:].unsqueeze(1).to_broadcast(
                                (128, nz, 132)))
                    scol[0] += ngath[P] * GW
                    nc.sync.dma_start(
                        out=ERAW[ds(P * CORE_N, CORE_N), 0:132].rearrange(
                            "(t p) c -> p t c", p=128),
                        in_=osb[:].rearrange("p (t c) -> p t c", c=132))

                    if P % 2 == 1:
                        am = pacc.tile([128, NB * 128], BF16, tag=f"am{m}",
                                       name=f"am{m}")
                        ams[m] = am
                        ita = pix.tile([128, NCHO * GW], I16, tag="ita")
                        nc.sync.dma_start(
                            out=ita[:],
                            in_=IDXW[:, ds(aln_off + (P - 1) * NCHO * GW,
                                           NCHO * GW)])
                        itb = pix.tile([128, NCHO * GW], I16, tag="itb")
                        nc.sync.dma_start(
                            out=itb[:],
                            in_=IDXW[:, ds(aln_off + P * NCHO * GW,
                                           NCHO * GW)])
                        for g in range(NCHO):
                            nblk = min(GT, NB - g * GT)
                            ni = min(GSZ, CORE_N - g * GSZ)
                            ra = prl.tile([128, GT * 256], BF16, tag="ra")
                            nc.gpsimd.dma_gather(
                                ra[:, ds(0, nblk * 256)].rearrange(
                                    "p (t e) -> p t e", e=256),
                                ERAW[ds((P - 1) * CORE_N, CORE_N), :],
                                ita[:, ds(g * GW, ni // 16)], ni, ni, 256,
                                queue_num=nextq())
                            rb = prl.tile([128, GT * 256], BF16, tag="rb")
                            nc.gpsimd.dma_gather(
                                rb[:, ds(0, nblk * 256)].rearrange(
                                    "p (t e) -> p t e", e=256),
                                ERAW[ds(P * CORE_N, CORE_N), :],
                                itb[:, ds(g * GW, ni // 16)], ni, ni, 256,
                                queue_num=nextq())
                            rav = ra[:].rearrange("p (t e) -> p t e", e=256)
                            rbv = rb[:].rearrange("p (t e) -> p t e", e=256)
                            ms = prl.tile([128, GT * 128], BF16, tag="ms")
                            nc.vector.tensor_tensor(
                                out=ms[:, 0:nblk * 128].rearrange(
                                    "p (t c) -> p t c", c=128),
                                in0=rav[:, 0:nblk, 0:128],
                                in1=rbv[:, 0:nblk, 0:128], op=OP.add)
                            ss = prl.tile([128, GT * 4], F32, tag="ss")
                            nc.vector.tensor_tensor(
                                out=ss[:, 0:nblk * 4].rearrange(
                                    "p (t h) -> p t h", h=4),
                                in0=rav[:, 0:nblk, 128:132],
                                in1=rbv[:, 0:nblk, 128:132], op=OP.add)
                            dn = prl.tile([128, GT * 4], F32, tag="dn")
                            nc.vector.tensor_scalar_add(
                                out=dn[:, 0:nblk * 4],
                                in0=ss[:, 0:nblk * 4], scalar1=1e-6)
                            rc = prl.tile([128, GT * 4], F32, tag="rc")
                            nc.vector.reciprocal(out=rc[:, 0:nblk * 4],
                                                 in_=dn[:, 0:nblk * 4])
                            nc.vector.tensor_tensor(
                                out=am[:, ds(g * GT * 128,
                                             nblk * 128)].rearrange(
                                    "p (t h c) -> p t h c", h=4, c=32),
                                in0=ms[:, 0:nblk * 128].rearrange(
                                    "p (t h c) -> p t h c", h=4, c=32),
                                in1=rc[:, 0:nblk * 4].rearrange(
                                    "p (t h) -> p t h", h=4).unsqueeze(
                                    3).to_broadcast((128, nblk, 4, 32)),
                                op=OP.mult)
                            if P == NP - 1:
                                for sub in range(0, nblk, 4):
                                    nb2 = min(4, nblk - sub)
                                    emit_s3(g * GT + sub, nb2, ams)

    nc.compile()
    # Align SWDGE queue assignment with the Tile-assigned DMASW sem lanes:
    # all DMAs on lane DMASW<k> must use one queue, and lanes are assigned
    # round-robin in SCHEDULED order (which differs from emission order).
    for f in nc.m.functions:
        for bb in f.blocks:
            for inst in bb.instructions:
                if type(inst).__name__ == "InstDMAGatherAnt":
                    inst.queue_num = (inst.bass_scheduled_proc - 11) % 4
    return nc


def kernel(feats, edge_index, W, b, attn, rel_attn_l, rel_attn_r,
           rel_attn_bias, _trace=False):
    plan, in_maps = _prep_host(feats, edge_index, W, attn, rel_attn_l,
                               rel_attn_r, rel_attn_bias)
    key = tuple(plan["Tu"].ravel())
    if key not in _CACHE:
        _CACHE.clear()
        _CACHE[key] = _build(plan)
    nc = _CACHE[key]
    res = run_bass_kernel_spmd(nc, in_maps, core_ids=list(range(NCORES)),
                               trace=_trace)
    parts = []
    for k in range(NCORES):
        rows = min(CORE_N, N - CORE_N * k)
        parts.append(np.asarray(res.results[k]["OUT"][:rows],
                                dtype=np.float32))
    out = np.concatenate(parts, axis=0)
    if _trace:
        kernel._last_exec_ns = res.exec_time_ns
    return out
